# revision 7
# baseline (speedup 1.0000x reference)
"""Distributed Trainium2 kernel for the additive-attention alignment predictor.

Math: score[b,t,u] = sum_h w_h * tanh(ep[b,t,h] + dp[b,u,h]);  out = softmax_u(score)
  where ep = enc @ W_enc (bias folded into dp), dp = dec @ W_dec + b_enc + b_dec.
  (b_score is dropped: softmax is shift-invariant.)

Key trick: tanh(z) on z in [-5.8, 5.8] is replaced by the separable expansion
  tanh(z) ~= c1*z + c3*z^3 + sum_f c_f sin(a_f z)
  sin(a(x+y)) = sin(ax)cos(ay) + cos(ax)sin(ay);  z^3 expands into x^i y^j products,
so the whole [T,U,H] contraction becomes TensorEngine matmuls over an expanded
contraction axis, with only O((T+U)*H) ScalarEngine work for the feature planes.

The device Sin table is only accurate for |arg| <= ~3.3, so sin/cos planes are
built via half-angle evaluation (args <= 1.73, cos arg <= 3.3) and a double-angle
ladder on the Vector engine for the high frequencies {2a, 4a, 2b, 4b}.
Max fit error 4.4e-3 (8.5e-4 for |z|<3); end-to-end softmax rel err ~3.5e-4
validated in numpy with this exact arithmetic.

Sharding: data-parallel over (B, T/2): core c handles batch c//2, t-half c%2.
No cross-core communication.
"""

import math

import numpy as np

import concourse.bass as bass
import concourse.tile as tile
from concourse import bacc, mybir
from concourse.bass_utils import run_bass_kernel_spmd

# Problem shapes (hardcoded per spec)
B, T, U = 4, 800, 150
D, H = 512, 256
NCORES = 8
TPC = T * B // NCORES  # 400 t-rows per core
P = 128
KT = D // P  # 4 contraction tiles for the projections
HT = H // P  # 2 h-tiles
EPOFF, DPOFF, EPDPW = 0, TPC, TPC + U  # ep|dp concat layout in sbuf
TBLK = [(i * P, min(P, TPC - i * P)) for i in range((TPC + P - 1) // P)]

# Fitted expansion: tanh(z) ~= C1*z + C3*z^3 + sum c_f sin(f z), |z|<=5.8
FD, FA, FB = 1.210016, 0.789398, 1.085813
C1, C3 = 0.456703, -0.009294
# frequency name -> (coefficient); ladder: 2a,4a from a; 2b,4b from b
CFREQ = {
    "d": 0.225962, "a": -0.011768, "b": 0.051665,
    "2a": 0.018029, "4a": 0.014671, "2b": 0.061241, "4b": 0.003115,
}
FREQ_ORDER = ["d", "a", "b", "2a", "4a", "2b", "4b"]
# wbt scale-table columns: [C1, 3*C3, C3, c_d, c_a, c_b, c_2a, c_4a, c_2b, c_4b]
WBT_COLS = [C1, 3 * C3, C3] + [CFREQ[f] for f in FREQ_ORDER]
NWB = len(WBT_COLS)

F32 = mybir.dt.float32
AF = mybir.ActivationFunctionType
ALU = mybir.AluOpType


def _build_graph():
    nc = bacc.Bacc()
    enc_x = nc.declare_dram_parameter("enc_t", [D, TPC], F32, isOutput=False)
    dec_x = nc.declare_dram_parameter("dec_t", [D, U], F32, isOutput=False)
    we_x = nc.declare_dram_parameter("w_enc", [D, H], F32, isOutput=False)
    wd_x = nc.declare_dram_parameter("w_dec", [D, H], F32, isOutput=False)
    bias_x = nc.declare_dram_parameter("bias2", [P, HT], F32, isOutput=False)
    wbt_x = nc.declare_dram_parameter("wbt", [P, HT, NWB], F32, isOutput=False)
    out_x = nc.declare_dram_parameter("out", [TPC, U], F32, isOutput=True)

    with tile.TileContext(nc) as tc:
        with (
            tc.tile_pool(name="const", bufs=1) as const,
            tc.tile_pool(name="tmp", bufs=2) as tmp,
            tc.tile_pool(name="soft", bufs=2) as soft,
            tc.tile_pool(name="ppsum", bufs=1, space="PSUM") as ppsum,
            tc.tile_pool(name="spsum", bufs=1, space="PSUM") as spsum,
        ):
            # ---- inputs to SBUF
            enc_sb = const.tile([P, KT, TPC], F32)
            nc.sync.dma_start(out=enc_sb, in_=enc_x[:].rearrange("(k p) t -> p k t", p=P))
            dec_sb = const.tile([P, KT, U], F32)
            nc.sync.dma_start(out=dec_sb, in_=dec_x[:].rearrange("(k p) u -> p k u", p=P))
            we_sb = const.tile([P, KT, H], F32)
            nc.sync.dma_start(out=we_sb, in_=we_x[:].rearrange("(k p) h -> p k h", p=P))
            wd_sb = const.tile([P, KT, H], F32)
            nc.sync.dma_start(out=wd_sb, in_=wd_x[:].rearrange("(k p) h -> p k h", p=P))
            bias_sb = const.tile([P, HT], F32)
            nc.sync.dma_start(out=bias_sb, in_=bias_x[:])
            wbt_sb = const.tile([P, HT, NWB], F32)
            nc.sync.dma_start(out=wbt_sb, in_=wbt_x[:])

            # ---- projections -> epdp[p, m, 0:TPC]=ep, [.., TPC:TPC+U]=dp(+biases)
            epdp = const.tile([P, HT, EPDPW], F32)
            for m in range(HT):
                ps_ep = ppsum.tile([P, TPC], F32, name=f"ps_ep{m}")
                for k in range(KT):
                    nc.tensor.matmul(
                        ps_ep,
                        lhsT=we_sb[:, k, m * P : (m + 1) * P],
                        rhs=enc_sb[:, k, :],
                        start=(k == 0),
                        stop=(k == KT - 1),
                    )
                nc.vector.tensor_copy(epdp[:, m, EPOFF : EPOFF + TPC], ps_ep)
                ps_dp = ppsum.tile([P, U], F32, name=f"ps_dp{m}")
                for k in range(KT):
                    nc.tensor.matmul(
                        ps_dp,
                        lhsT=wd_sb[:, k, m * P : (m + 1) * P],
                        rhs=dec_sb[:, k, :],
                        start=(k == 0),
                        stop=(k == KT - 1),
                    )
                nc.vector.tensor_scalar_add(
                    out=epdp[:, m, DPOFF : DPOFF + U],
                    in0=ps_dp,
                    scalar1=bias_sb[:, m : m + 1],
                )

            ones_a = const.tile([P, P], F32)
            nc.vector.memset(ones_a, 1.0)
            ones_u = const.tile([P, U], F32)
            nc.vector.memset(ones_u, 1.0)
            halfpi = const.tile([P, 1], F32)
            nc.vector.memset(halfpi, math.pi / 2)

            # ---- feature planes on the ep|dp concat (all [P, HT, EPDPW])
            # squares plane: x^2 | y^2
            sq = const.tile([P, HT, EPDPW], F32)
            nc.scalar.activation(out=sq, in_=epdp, func=AF.Square, scale=1.0)
            # y^3 (dp side only)
            v3 = const.tile([P, HT, U], F32)
            nc.vector.tensor_tensor(
                out=v3,
                in0=epdp[:, :, DPOFF : DPOFF + U],
                in1=sq[:, :, DPOFF : DPOFF + U],
                op=ALU.mult,
            )

            # sin/cos planes per frequency via half-angle + double-angle ladder
            sin_p, cos_p = {}, {}
            for nm, f in (("d", FD), ("a", FA), ("b", FB)):
                sh = tmp.tile([P, HT, EPDPW], F32, name="sh")
                nc.scalar.activation(out=sh, in_=epdp, func=AF.Sin, scale=float(f / 2))
                ch = tmp.tile([P, HT, EPDPW], F32, name="ch")
                nc.scalar.activation(
                    out=ch, in_=epdp, func=AF.Sin, scale=float(f / 2), bias=halfpi[:, :]
                )
                sin_p[nm] = const.tile([P, HT, EPDPW], F32, name=f"sin_{nm}")
                nc.vector.scalar_tensor_tensor(
                    out=sin_p[nm], in0=sh, scalar=2.0, in1=ch,
                    op0=ALU.mult, op1=ALU.mult,
                )
                shsq = tmp.tile([P, HT, EPDPW], F32, name="shsq")
                nc.scalar.activation(out=shsq, in_=sh, func=AF.Square, scale=1.0)
                cos_p[nm] = const.tile([P, HT, EPDPW], F32, name=f"cos_{nm}")
                nc.vector.tensor_scalar(
                    out=cos_p[nm], in0=shsq, scalar1=-2.0, scalar2=1.0,
                    op0=ALU.mult, op1=ALU.add,
                )
            for src, dst in (("a", "2a"), ("2a", "4a"), ("b", "2b"), ("2b", "4b")):
                sin_p[dst] = const.tile([P, HT, EPDPW], F32, name=f"sin_{dst}")
                nc.vector.scalar_tensor_tensor(
                    out=sin_p[dst], in0=sin_p[src], scalar=2.0, in1=cos_p[src],
                    op0=ALU.mult, op1=ALU.mult,
                )
                ssq = tmp.tile([P, HT, EPDPW], F32, name="shsq")
                nc.scalar.activation(out=ssq, in_=sin_p[src], func=AF.Square, scale=1.0)
                cos_p[dst] = const.tile([P, HT, EPDPW], F32, name=f"cos_{dst}")
                nc.vector.tensor_scalar(
                    out=cos_p[dst], in0=ssq, scalar1=-2.0, scalar2=1.0,
                    op0=ALU.mult, op1=ALU.add,
                )

            # ---- B-side planes: dp-part scaled by (coef * w_h)
            # (A_sel, B_source_ap_or_None(ones), wbt column)
            def dp_part(t):
                return t[:, :, DPOFF - EPOFF : DPOFF - EPOFF + U] if t.shape[2] == EPDPW else t

            b_tiles = []

            def make_b(src, col, name):
                bt = const.tile([P, HT, U], F32, name=name)
                for m in range(HT):
                    nc.vector.tensor_scalar_mul(
                        out=bt[:, m, :],
                        in0=(ones_u if src is None else dp_part(src)[:, m, :]),
                        scalar1=wbt_sb[:, m, col : col + 1],
                    )
                return bt

            b_one = make_b(None, 0, "b_one")      # (c1 w) * 1
            b_y1 = make_b(epdp, 0, "b_y1")        # (c1 w) * y
            b_y3c = make_b(epdp, 1, "b_y3c")      # (3c3 w) * y
            b_sq = make_b(sq, 1, "b_sq")          # (3c3 w) * y^2
            b_cu = make_b(v3, 2, "b_cu")          # (c3 w) * y^3
            b_sin, b_cos = {}, {}
            for i, nm in enumerate(FREQ_ORDER):
                b_sin[nm] = make_b(sin_p[nm], 3 + i, f"b_sin_{nm}")
                b_cos[nm] = make_b(cos_p[nm], 3 + i, f"b_cos_{nm}")

            # ---- matmul pair list: (A plane tile or "ones", B tile)
            pairs = [(epdp, b_one), ("ones", b_y1), (sq, b_y3c), (epdp, b_sq), ("ones", b_cu)]
            for nm in FREQ_ORDER:
                pairs.append((sin_p[nm], b_cos[nm]))
                pairs.append((cos_p[nm], b_sin[nm]))

            sp = [spsum.tile([P, U], F32, name=f"sp{tb}") for tb in range(len(TBLK))]
            n_mm = 2 * len(pairs)
            for tb, (t0, pn) in enumerate(TBLK):
                i = 0
                for a_t, b_t in pairs:
                    for m in range(HT):
                        lhsT = (
                            ones_a[:, :pn]
                            if isinstance(a_t, str)
                            else a_t[:, m, EPOFF + t0 : EPOFF + t0 + pn]
                        )
                        nc.tensor.matmul(
                            sp[tb][:pn, :],
                            lhsT=lhsT,
                            rhs=b_t[:, m, :],
                            start=(i == 0),
                            stop=(i == n_mm - 1),
                        )
                        i += 1

                # ---- softmax over u for this t-block, then DMA out
                nmax = soft.tile([P, 1], F32, name=f"nmax{tb}")
                nc.vector.tensor_reduce(
                    out=nmax[:pn], in_=sp[tb][:pn, :], axis=mybir.AxisListType.X,
                    op=ALU.max, negate=True,
                )
                expt = soft.tile([P, U], F32, name=f"expt{tb}")
                nc.scalar.activation(
                    out=expt[:pn], in_=sp[tb][:pn, :], func=AF.Exp,
                    bias=nmax[:pn], scale=1.0,
                )
                ssum = soft.tile([P, 1], F32, name=f"ssum{tb}")
                nc.vector.tensor_reduce(
                    out=ssum[:pn], in_=expt[:pn, :], axis=mybir.AxisListType.X,
                    op=ALU.add,
                )
                nc.vector.reciprocal(out=ssum[:pn], in_=ssum[:pn])
                outt = soft.tile([P, U], F32, name=f"outt{tb}")
                nc.vector.tensor_scalar_mul(out=outt[:pn], in0=expt[:pn, :], scalar1=ssum[:pn])
                nc.sync.dma_start(out=out_x[t0 : t0 + pn, :], in_=outt[:pn, :])

    nc.finalize()
    return nc


_NC_CACHE = None


def kernel(**inputs: np.ndarray) -> np.ndarray:
    global _NC_CACHE
    enc = np.asarray(inputs["encoder_out"], dtype=np.float32)
    dec = np.asarray(inputs["decoder_out"], dtype=np.float32)
    w_enc = np.ascontiguousarray(inputs["W_enc"], dtype=np.float32)
    b_enc = np.asarray(inputs["b_enc"], dtype=np.float32)
    w_dec = np.ascontiguousarray(inputs["W_dec"], dtype=np.float32)
    b_dec = np.asarray(inputs["b_dec"], dtype=np.float32)
    w_score = np.asarray(inputs["w_score"], dtype=np.float32)
    # b_score dropped: softmax(x + c) == softmax(x)

    bias2 = np.ascontiguousarray((b_enc + b_dec).reshape(HT, P).T)  # [P, HT]
    wbt = np.empty((P, HT, NWB), dtype=np.float32)
    for m in range(HT):
        wseg = w_score[m * P : (m + 1) * P]
        for j, c in enumerate(WBT_COLS):
            wbt[:, m, j] = np.float32(c) * wseg
    wbt = np.ascontiguousarray(wbt)

    in_maps = []
    for c in range(NCORES):
        b = c // (NCORES // B)
        t0 = (c % (NCORES // B)) * TPC
        in_maps.append(
            {
                "enc_t": np.ascontiguousarray(enc[b, t0 : t0 + TPC, :].T),
                "dec_t": np.ascontiguousarray(dec[b].T),
                "w_enc": w_enc,
                "w_dec": w_dec,
                "bias2": bias2,
                "wbt": wbt,
            }
        )

    if _NC_CACHE is None:
        _NC_CACHE = _build_graph()
    res = run_bass_kernel_spmd(_NC_CACHE, in_maps, core_ids=list(range(NCORES)))

    out = np.empty((B, T, U), dtype=np.float32)
    for c in range(NCORES):
        b = c // (NCORES // B)
        t0 = (c % (NCORES // B)) * TPC
        out[b, t0 : t0 + TPC, :] = res.results[c]["out"]
    return out


# revision 8
# speedup vs baseline: 1.2618x; 1.2618x over previous
"""Distributed Trainium2 kernel for the additive-attention alignment predictor.

Math: score[b,t,u] = sum_h w_h * tanh(ep[b,t,h] + dp[b,u,h]);  out = softmax_u(score)
  where ep = enc @ W_enc (bias folded into dp), dp = dec @ W_dec + b_enc + b_dec.
  (b_score is dropped: softmax is shift-invariant.)

Key trick: tanh(z) on z in [-5.8, 5.8] is replaced by the separable expansion
  tanh(z) ~= c1*z + c3*z^3 + sum_f c_f sin(a_f z)
  sin(a(x+y)) = sin(ax)cos(ay) + cos(ax)sin(ay);  z^3 expands into x^i y^j products,
so the whole [T,U,H] contraction becomes TensorEngine matmuls over an expanded
contraction axis, with only O((T+U)*H) ScalarEngine work for the feature planes.

The device Sin table is only accurate for |arg| <= ~3.3, so sin/cos planes are
built via half-angle evaluation (args <= 1.73, cos arg <= 3.3) and a double-angle
ladder on the Vector engine for the high frequencies {2a, 4a, 2b, 4b}.
Feature planes are bf16 (validated end-to-end rel err ~1.1e-3): bf16 matmuls are
single-pass with fast weight load, vs 2-pass HI/LO for fp32.

Sharding: data-parallel over (B, T/2): core c handles batch c//2, t-half c%2.
No cross-core communication.
"""

import math

import numpy as np

import concourse.bass as bass
import concourse.tile as tile
from concourse import bacc, mybir
from concourse.bass_utils import run_bass_kernel_spmd

# Problem shapes (hardcoded per spec)
B, T, U = 4, 800, 150
D, H = 512, 256
NCORES = 8
TPC = T * B // NCORES  # 400 t-rows per core
P = 128
KT = D // P  # 4 contraction tiles for the projections
HT = H // P  # 2 h-tiles
EPOFF, DPOFF, EPDPW = 0, TPC, TPC + U  # ep|dp concat layout in sbuf
TBLK = [(i * P, min(P, TPC - i * P)) for i in range((TPC + P - 1) // P)]

# Fitted expansion: tanh(z) ~= C1*z + C3*z^3 + sum c_f sin(f z), |z|<=5.8
FD, FA, FB = 1.210016, 0.789398, 1.085813
C1, C3 = 0.456703, -0.009294
CFREQ = {
    "d": 0.225962, "a": -0.011768, "b": 0.051665,
    "2a": 0.018029, "4a": 0.014671, "2b": 0.061241, "4b": 0.003115,
}
# pair order: direct freqs first, ladder-derived later (they depend on DVE chain)
FREQ_ORDER = ["d", "a", "b", "2a", "2b", "4a", "4b"]
# wbt scale-table columns: [C1, 3*C3, C3, c_f...]
WBT_COLS = [C1, 3 * C3, C3] + [CFREQ[f] for f in FREQ_ORDER]
NWB = len(WBT_COLS)

F32 = mybir.dt.float32
BF16 = mybir.dt.bfloat16
AF = mybir.ActivationFunctionType
ALU = mybir.AluOpType


def _build_graph():
    nc = bacc.Bacc()
    enc_x = nc.declare_dram_parameter("enc_t", [D, TPC], F32, isOutput=False)
    dec_x = nc.declare_dram_parameter("dec_t", [D, U], F32, isOutput=False)
    we_x = nc.declare_dram_parameter("w_enc", [D, H], F32, isOutput=False)
    wd_x = nc.declare_dram_parameter("w_dec", [D, H], F32, isOutput=False)
    bias_x = nc.declare_dram_parameter("bias2", [P, HT], F32, isOutput=False)
    wbt_x = nc.declare_dram_parameter("wbt", [P, HT, NWB], F32, isOutput=False)
    out_x = nc.declare_dram_parameter("out", [TPC, U], F32, isOutput=True)

    enc_v = enc_x[:].rearrange("(k p) t -> p k t", p=P)
    dec_v = dec_x[:].rearrange("(k p) u -> p k u", p=P)
    we_v = we_x[:].rearrange("(k p) h -> p k h", p=P)
    wd_v = wd_x[:].rearrange("(k p) h -> p k h", p=P)

    with tile.TileContext(nc) as tc:
        with (
            tc.tile_pool(name="const", bufs=1) as const,
            tc.tile_pool(name="tmp", bufs=2) as tmp,
            tc.tile_pool(name="soft", bufs=2) as soft,
            tc.tile_pool(name="ppsum", bufs=1, space="PSUM") as ppsum,
            tc.tile_pool(name="spsum", bufs=1, space="PSUM") as spsum,
        ):
            # ---- inputs to SBUF, sliced along k so projections start early
            enc_sb = const.tile([P, KT, TPC], F32)
            dec_sb = const.tile([P, KT, U], F32)
            we_sb = const.tile([P, KT, H], F32)
            wd_sb = const.tile([P, KT, H], F32)
            for k in range(KT):
                nc.sync.dma_start(out=we_sb[:, k, :], in_=we_v[:, k, :])
                nc.sync.dma_start(out=enc_sb[:, k, :], in_=enc_v[:, k, :])
                nc.sync.dma_start(out=wd_sb[:, k, :], in_=wd_v[:, k, :])
                nc.sync.dma_start(out=dec_sb[:, k, :], in_=dec_v[:, k, :])
            bias_sb = const.tile([P, HT], F32)
            nc.sync.dma_start(out=bias_sb, in_=bias_x[:])
            wbt_sb = const.tile([P, HT, NWB], F32)
            nc.sync.dma_start(out=wbt_sb, in_=wbt_x[:])

            # ---- projections -> epdp[p, m, 0:TPC]=ep, [.., TPC:TPC+U]=dp(+biases)
            epdp = const.tile([P, HT, EPDPW], F32)
            ps_ep = [ppsum.tile([P, TPC], F32, name=f"ps_ep{m}") for m in range(HT)]
            ps_dp = [ppsum.tile([P, U], F32, name=f"ps_dp{m}") for m in range(HT)]
            for k in range(KT):
                for m in range(HT):
                    nc.tensor.matmul(
                        ps_ep[m],
                        lhsT=we_sb[:, k, m * P : (m + 1) * P],
                        rhs=enc_sb[:, k, :],
                        start=(k == 0),
                        stop=(k == KT - 1),
                    )
                    nc.tensor.matmul(
                        ps_dp[m],
                        lhsT=wd_sb[:, k, m * P : (m + 1) * P],
                        rhs=dec_sb[:, k, :],
                        start=(k == 0),
                        stop=(k == KT - 1),
                    )
            for m in range(HT):
                nc.vector.tensor_copy(epdp[:, m, EPOFF : EPOFF + TPC], ps_ep[m])
                nc.vector.tensor_scalar_add(
                    out=epdp[:, m, DPOFF : DPOFF + U],
                    in0=ps_dp[m],
                    scalar1=bias_sb[:, m : m + 1],
                )

            ones_a = const.tile([P, P], BF16)
            nc.vector.memset(ones_a, 1.0)
            ones_u = const.tile([P, U], BF16)
            nc.vector.memset(ones_u, 1.0)
            halfpi = const.tile([P, 1], F32)
            nc.vector.memset(halfpi, math.pi / 2)

            # bf16 copy of epdp (A-side linear plane; also y for B scaling)
            epdp_bf = const.tile([P, HT, EPDPW], BF16)
            nc.vector.tensor_copy(epdp_bf, epdp)

            # squares plane x^2|y^2 and y^3 (dp side)
            sq = const.tile([P, HT, EPDPW], BF16)
            nc.scalar.activation(out=sq, in_=epdp, func=AF.Square, scale=1.0)
            v3 = const.tile([P, HT, U], BF16)
            nc.vector.tensor_tensor(
                out=v3,
                in0=epdp_bf[:, :, DPOFF : DPOFF + U],
                in1=sq[:, :, DPOFF : DPOFF + U],
                op=ALU.mult,
            )

            # sin/cos planes per frequency: half-angle on ACT, ladder on DVE
            sin_p, cos_p = {}, {}
            for nm, f in (("d", FD), ("a", FA), ("b", FB)):
                sh = tmp.tile([P, HT, EPDPW], BF16, name="sh")
                nc.scalar.activation(out=sh, in_=epdp, func=AF.Sin, scale=float(f / 2))
                ch = tmp.tile([P, HT, EPDPW], BF16, name="ch")
                nc.scalar.activation(
                    out=ch, in_=epdp, func=AF.Sin, scale=float(f / 2), bias=halfpi[:, :]
                )
                sin_p[nm] = const.tile([P, HT, EPDPW], BF16, name=f"sin_{nm}")
                nc.vector.scalar_tensor_tensor(
                    out=sin_p[nm], in0=sh, scalar=2.0, in1=ch,
                    op0=ALU.mult, op1=ALU.mult,
                )
                shsq = tmp.tile([P, HT, EPDPW], BF16, name="shsq")
                nc.scalar.activation(out=shsq, in_=sh, func=AF.Square, scale=1.0)
                cos_p[nm] = const.tile([P, HT, EPDPW], BF16, name=f"cos_{nm}")
                nc.vector.tensor_scalar(
                    out=cos_p[nm], in0=shsq, scalar1=-2.0, scalar2=1.0,
                    op0=ALU.mult, op1=ALU.add,
                )
            for src, dst in (("a", "2a"), ("2a", "4a"), ("b", "2b"), ("2b", "4b")):
                sin_p[dst] = const.tile([P, HT, EPDPW], BF16, name=f"sin_{dst}")
                nc.vector.scalar_tensor_tensor(
                    out=sin_p[dst], in0=sin_p[src], scalar=2.0, in1=cos_p[src],
                    op0=ALU.mult, op1=ALU.mult,
                )
                nsq = tmp.tile([P, HT, EPDPW], BF16, name="nsq")
                nc.vector.scalar_tensor_tensor(
                    out=nsq, in0=sin_p[src], scalar=-2.0, in1=sin_p[src],
                    op0=ALU.mult, op1=ALU.mult,
                )
                cos_p[dst] = const.tile([P, HT, EPDPW], BF16, name=f"cos_{dst}")
                nc.vector.tensor_scalar_add(out=cos_p[dst], in0=nsq, scalar1=1.0)

            # ---- B-side planes: dp-part scaled by (coef * w_h), bf16
            def dp_part(t):
                return t[:, :, DPOFF : DPOFF + U] if t.shape[2] == EPDPW else t

            def make_b(src, col, name):
                bt = const.tile([P, HT, U], BF16, name=name)
                for m in range(HT):
                    nc.vector.tensor_scalar_mul(
                        out=bt[:, m, :],
                        in0=(ones_u if src is None else dp_part(src)[:, m, :]),
                        scalar1=wbt_sb[:, m, col : col + 1],
                    )
                return bt

            b_one = make_b(None, 0, "b_one")        # (c1 w) * 1
            b_y1 = make_b(epdp_bf, 0, "b_y1")       # (c1 w) * y
            b_y3c = make_b(epdp_bf, 1, "b_y3c")     # (3c3 w) * y
            b_sq = make_b(sq, 1, "b_sq")            # (3c3 w) * y^2
            b_cu = make_b(v3, 2, "b_cu")            # (c3 w) * y^3
            b_sin, b_cos = {}, {}
            for i, nm in enumerate(FREQ_ORDER):
                b_sin[nm] = make_b(sin_p[nm], 3 + i, f"b_sin_{nm}")
                b_cos[nm] = make_b(cos_p[nm], 3 + i, f"b_cos_{nm}")

            # ---- matmul pair list: (A plane tile or "ones", B tile)
            pairs = [(epdp_bf, b_one), ("ones", b_y1), (sq, b_y3c), (epdp_bf, b_sq), ("ones", b_cu)]
            for nm in FREQ_ORDER:
                pairs.append((sin_p[nm], b_cos[nm]))
                pairs.append((cos_p[nm], b_sin[nm]))

            sp = [spsum.tile([P, U], F32, name=f"sp{tb}") for tb in range(len(TBLK))]
            n_mm = 2 * len(pairs)
            for tb, (t0, pn) in enumerate(TBLK):
                i = 0
                for a_t, b_t in pairs:
                    for m in range(HT):
                        lhsT = (
                            ones_a[:, :pn]
                            if isinstance(a_t, str)
                            else a_t[:, m, EPOFF + t0 : EPOFF + t0 + pn]
                        )
                        nc.tensor.matmul(
                            sp[tb][:pn, :],
                            lhsT=lhsT,
                            rhs=b_t[:, m, :],
                            start=(i == 0),
                            stop=(i == n_mm - 1),
                        )
                        i += 1

                # ---- softmax over u for this t-block, then DMA out
                nmax = soft.tile([P, 1], F32, name=f"nmax{tb}")
                nc.vector.tensor_reduce(
                    out=nmax[:pn], in_=sp[tb][:pn, :], axis=mybir.AxisListType.X,
                    op=ALU.max, negate=True,
                )
                expt = soft.tile([P, U], F32, name=f"expt{tb}")
                nc.scalar.activation(
                    out=expt[:pn], in_=sp[tb][:pn, :], func=AF.Exp,
                    bias=nmax[:pn], scale=1.0,
                )
                ssum = soft.tile([P, 1], F32, name=f"ssum{tb}")
                nc.vector.tensor_reduce(
                    out=ssum[:pn], in_=expt[:pn, :], axis=mybir.AxisListType.X,
                    op=ALU.add,
                )
                nc.vector.reciprocal(out=ssum[:pn], in_=ssum[:pn])
                outt = soft.tile([P, U], F32, name=f"outt{tb}")
                nc.vector.tensor_scalar_mul(out=outt[:pn], in0=expt[:pn, :], scalar1=ssum[:pn])
                nc.sync.dma_start(out=out_x[t0 : t0 + pn, :], in_=outt[:pn, :])

    nc.finalize()
    return nc


_NC_CACHE = None


def kernel(**inputs: np.ndarray) -> np.ndarray:
    global _NC_CACHE
    enc = np.asarray(inputs["encoder_out"], dtype=np.float32)
    dec = np.asarray(inputs["decoder_out"], dtype=np.float32)
    w_enc = np.ascontiguousarray(inputs["W_enc"], dtype=np.float32)
    b_enc = np.asarray(inputs["b_enc"], dtype=np.float32)
    w_dec = np.ascontiguousarray(inputs["W_dec"], dtype=np.float32)
    b_dec = np.asarray(inputs["b_dec"], dtype=np.float32)
    w_score = np.asarray(inputs["w_score"], dtype=np.float32)
    # b_score dropped: softmax(x + c) == softmax(x)

    bias2 = np.ascontiguousarray((b_enc + b_dec).reshape(HT, P).T)  # [P, HT]
    wbt = np.empty((P, HT, NWB), dtype=np.float32)
    for m in range(HT):
        wseg = w_score[m * P : (m + 1) * P]
        for j, c in enumerate(WBT_COLS):
            wbt[:, m, j] = np.float32(c) * wseg
    wbt = np.ascontiguousarray(wbt)

    in_maps = []
    for c in range(NCORES):
        b = c // (NCORES // B)
        t0 = (c % (NCORES // B)) * TPC
        in_maps.append(
            {
                "enc_t": np.ascontiguousarray(enc[b, t0 : t0 + TPC, :].T),
                "dec_t": np.ascontiguousarray(dec[b].T),
                "w_enc": w_enc,
                "w_dec": w_dec,
                "bias2": bias2,
                "wbt": wbt,
            }
        )

    if _NC_CACHE is None:
        _NC_CACHE = _build_graph()
    res = run_bass_kernel_spmd(_NC_CACHE, in_maps, core_ids=list(range(NCORES)))

    out = np.empty((B, T, U), dtype=np.float32)
    for c in range(NCORES):
        b = c // (NCORES // B)
        t0 = (c % (NCORES // B)) * TPC
        out[b, t0 : t0 + TPC, :] = res.results[c]["out"]
    return out
